# revision 15
# baseline (speedup 1.0000x reference)
"""Multi-head attention (B=4, S=1024, D=1024, H=16) on 8 TRN2 NeuronCores.

Sharding: data parallel on batch (4) x tensor parallel on heads (2 groups of
8 heads).  Core c handles batch c//2 and heads (c%2)*8 .. (c%2)*8+7.

Per-core dataflow (everything in "transposed" space so no on-device
transposes are needed):
  QT [512,1024] (d_out on partitions), KT likewise, V [1024,512] natural.
  V2 [k,h,65] = [V*mask | mask]  (65th column -> masked row-sums via matmul)
  scoresT[k,q] = KT_h.T @ QT_h   (K=64 contraction, head pairs row-packed at
  tile positions (0,0)/(64,0))
  pT = exp(scoresT/8)            (no max subtraction; scores are O(1))
  attnV psum[0:65] = V2_h.T @ pT (rows 0:64 numerator^T, row 64 denominator)
  normalize: fast reciprocal + ones-matmul partition broadcast, software
  pipelined one iteration behind so it stays off the PE critical path
  Wo: out[q,o] partial = CT.T @ WoT_loc ; host adds the two head-group halves.

Perf structure (measured on HW):
 - all matmul operands bfloat16: fp32 HIGH-mode matmuls trigger a sustained
   50% activity throttle; 16-bit operands run throttle-free.
 - back-to-back matmuls into the SAME psum bank serialize at ~379ns (the
   ~173ns psum drain can't overlap); alternating banks streams at the full
   216ns/matmul (2.4GHz) rate.  So: projections split each 8-chunk
   contraction into even/odd psum banks (combined during evacuation), the
   attention loop interleaves score and attnV matmuls, and Wo groups
   alternate two banks.
 - score psum tiles are [128,1024] spanning 2 banks (2 matmuls each) so one
   activation instruction computes exp for a kc-pair: 64 instead of 128
   Act instructions (Act is the co-critical engine in attention).
 - psum pools are phase-scoped (projections release their 8-bank pool
   before the attention pools open).
 - DMA issue is round-robined across SP/Activation/GpSimd queues, w/x
   chunk loads interleaved in consumption order; output is fp16 (host
   upcasts) to halve the output-DMA tail.
End-to-end rel err ~5e-3 (gate 2e-2).
"""
import sys

if '/opt/trn_rl_repo' not in sys.path:
    sys.path.insert(0, '/opt/trn_rl_repo')

import numpy as np

P = 128
B, S, D = 4, 1024, 1024
DL = 512          # local d_out (8 heads x 64)
H = 8             # local heads
E = 64            # head dim
IC = D // P       # 8 contraction chunks for projections
KC = S // P       # 8 key-position chunks
T4 = DL // P      # 4 tiles holding QT/KT/CT rows
NQ = 512          # matmul moving free dim
N_CORES = 8

_prog_cache = {}

MM_DTYPE = 'bf16'   # matmul operand dtype: 'fp16' or 'bf16'


def build_program(recip_mode='fast', mm_dtype=None):
    import concourse.tile as tile
    from concourse import bacc, mybir

    F32 = mybir.dt.float32
    F16 = (mybir.dt.bfloat16 if (mm_dtype or MM_DTYPE) == 'bf16'
           else mybir.dt.float16)
    OUT16 = mybir.dt.float16
    EXP = mybir.ActivationFunctionType.Exp
    MULT = mybir.AluOpType.mult
    ADD = mybir.AluOpType.add

    nc = bacc.Bacc("TRN2", target_bir_lowering=False, debug=False,
                   enable_asserts=False, num_devices=N_CORES)

    xtq = nc.dram_tensor("xtq", (D, S), F16, kind="ExternalInput").ap()
    xtk = nc.dram_tensor("xtk", (D, S), F16, kind="ExternalInput").ap()
    xtv = nc.dram_tensor("xtv", (D, S), F16, kind="ExternalInput").ap()
    wq = nc.dram_tensor("wq", (D, DL), F16, kind="ExternalInput").ap()
    wk = nc.dram_tensor("wk", (D, DL), F16, kind="ExternalInput").ap()
    wv = nc.dram_tensor("wv", (D, DL), F16, kind="ExternalInput").ap()
    wo = nc.dram_tensor("wo", (DL, D), F16, kind="ExternalInput").ap()
    maskd = nc.dram_tensor("maskd", (P, KC), F32, kind="ExternalInput").ap()
    out = nc.dram_tensor("out", (S, D), OUT16, kind="ExternalOutput").ap()

    xtq_c = xtq.rearrange("(ic p) s -> ic p s", p=P)
    xtk_c = xtk.rearrange("(ic p) s -> ic p s", p=P)
    xtv_c = xtv.rearrange("(ic p) s -> ic p s", p=P)
    wq_c = wq.rearrange("(ic p) o -> ic p o", p=P)
    wk_c = wk.rearrange("(ic p) o -> ic p o", p=P)
    wv_c = wv.rearrange("(ic p) o -> ic p o", p=P)
    wo_c = wo.rearrange("(t p) o -> t p o", p=P)

    with tile.TileContext(nc) as tc:
        with tc.tile_pool(name="xt", bufs=12) as xt_pool, \
             tc.tile_pool(name="wp", bufs=10) as w_pool, \
             tc.tile_pool(name="qk", bufs=8) as qk_pool, \
             tc.tile_pool(name="v2p", bufs=8) as v2_pool, \
             tc.tile_pool(name="pp", bufs=8) as p_pool, \
             tc.tile_pool(name="ctp", bufs=4) as ct_pool, \
             tc.tile_pool(name="sm", bufs=2) as small, \
             tc.tile_pool(name="rbp", bufs=3) as rb_pool, \
             tc.tile_pool(name="ob", bufs=3) as out_pool:

            # ---- constants / small inputs ----
            mask_sb = small.tile([P, KC], F32, tag="mask")
            nc.sync.dma_start(mask_sb[:], maskd[:])
            ones_f = small.tile([1, E], F32, tag="ones_f")
            nc.gpsimd.memset(ones_f[:], 1.0)
            ones_r = small.tile([1, E], F16, tag="ones_r")
            nc.vector.tensor_copy(ones_r[:], ones_f[:])

            # dma_start executes on the issuing engine's queue (~680ns
            # each); round-robin across three hwdge-capable engines so
            # loads don't serialize into ~12us of issue latency.
            dma_engines = [nc.sync, nc.scalar, nc.gpsimd]
            dma_rr = [0]

            def dma_load(dst, src):
                eng = dma_engines[dma_rr[0] % len(dma_engines)]
                dma_rr[0] += 1
                eng.dma_start(dst, src)

            def load_chunks_interleaved(w_dram, x_dram):
                w_tiles, x_tiles = [], []
                for ic in range(IC):
                    wt = w_pool.tile([P, DL], F16, tag="wp", name=f"w{ic}")
                    dma_load(wt[:], w_dram[ic])
                    w_tiles.append(wt)
                    xt = xt_pool.tile([P, S], F16, tag="xt", name=f"x{ic}")
                    dma_load(xt[:], x_dram[ic])
                    x_tiles.append(xt)
                return w_tiles, x_tiles

            qt = [qk_pool.tile([P, S], F16, tag="qk", name=f"qt{i}")
                  for i in range(T4)]
            kt = [qk_pool.tile([P, S], F16, tag="qk", name=f"kt{i}")
                  for i in range(T4)]

            # ---- projections: psum pool scoped to this phase ----
            # Two independent accumulation chains interleave into two psum
            # banks so consecutive PE matmuls never hit the same bank (a
            # same-bank back-to-back matmul serializes on the ~173ns psum
            # drain; alternating banks streams at the full 216ns rate).
            with tc.tile_pool(name="psP", bufs=4, space="PSUM") as psP:
                # Q^T / K^T: interleave the sc=0 / sc=1 chains per t.
                for x_c, w_c, dest in ((xtq_c, wq_c, qt), (xtk_c, wk_c, kt)):
                    w_sb, x_sb = load_chunks_interleaved(w_c, x_c)
                    for t in range(T4):
                        psAB = [psP.tile([P, NQ], F32, tag="psP",
                                         name=f"pj{t}_{i}")
                                for i in range(2)]
                        for ic in range(IC):
                            for sc in range(2):
                                nc.tensor.matmul(
                                    psAB[sc][:],
                                    w_sb[ic][:, t * P:(t + 1) * P],
                                    x_sb[ic][:, sc * NQ:(sc + 1) * NQ],
                                    start=(ic == 0), stop=(ic == IC - 1))
                        for sc in range(2):
                            nc.vector.tensor_copy(
                                dest[t][:, sc * NQ:(sc + 1) * NQ],
                                psAB[sc][:])

                # V projection -> V2 = [V*mask | mask]; interleave
                # even/odd skc chains.
                w_sb, x_sb = load_chunks_interleaved(wv_c, xtv_c)
                v2 = [None] * KC
                for skc0 in range(0, KC, 2):
                    psAB = [psP.tile([P, NQ], F32, tag="psP",
                                     name=f"pv{skc0}_{i}")
                            for i in range(2)]
                    for ic in range(IC):
                        for j in range(2):
                            nc.tensor.matmul(
                                psAB[j][:],
                                x_sb[ic][:, (skc0 + j) * P:(skc0 + j + 1) * P],
                                w_sb[ic][:],
                                start=(ic == 0), stop=(ic == IC - 1))
                    for j in range(2):
                        skc = skc0 + j
                        v2t = v2_pool.tile([P, H, E + 1], F16, tag="v2")
                        nc.vector.tensor_scalar_mul(
                            v2t[:, :, 0:E],
                            psAB[j][:].rearrange("p (h e) -> p h e", h=H),
                            mask_sb[:, skc:skc + 1])
                        nc.vector.tensor_copy(
                            v2t[:, :, E:E + 1],
                            mask_sb[:, skc:skc + 1, None].to_broadcast(
                                (P, H, 1)))
                        v2[skc] = v2t

            # ---- Wo weights: loaded into freed xt slots during attention
            wo_sb = []
            for t in range(T4):
                wt = xt_pool.tile([P, D], F16, tag="xt", name=f"wo{t}")
                dma_load(wt[:], wo_c[t])
                wo_sb.append(wt)

            # ---- attention, qc-major; psum pools scoped to this phase ----
            ct = [ct_pool.tile([P, S], F16, tag="ct", name=f"ct{i}")
                  for i in range(T4)]

            with tc.tile_pool(name="psS", bufs=2, space="PSUM") as psS, \
                 tc.tile_pool(name="psO", bufs=2, space="PSUM") as psO, \
                 tc.tile_pool(name="psN", bufs=2, space="PSUM") as psN:

                def emit_recip(pso):
                    tmp = small.tile([1, 3 * NQ], F32, tag="ntmp")
                    d_ = tmp[0:1, 0:NQ]
                    s_ = tmp[0:1, NQ:2 * NQ]
                    r_ = tmp[0:1, 2 * NQ:3 * NQ]
                    if recip_mode in ('accurate', 'fast'):
                        nc.vector.tensor_copy(d_, pso[E:E + 1, :])
                        if recip_mode == 'accurate':
                            nc.vector.reciprocal_approx_accurate(r_, d_, s_)
                        else:
                            nc.vector.reciprocal_approx_fast(r_, d_)
                    else:
                        nc.vector.reciprocal(r_, pso[E:E + 1, :])
                    recip_r = small.tile([1, NQ], F16, tag="recip_r")
                    nc.vector.tensor_copy(recip_r[:], r_)
                    return recip_r

                def emit_norm(h, qc, pso, recip_r):
                    psr = psN.tile([E, NQ], F32, tag="psN")
                    nc.tensor.matmul(psr[:], ones_r[:], recip_r[:],
                                     start=True, stop=True)
                    rb = rb_pool.tile([E, NQ], F32, tag="rb")
                    nc.vector.tensor_copy(rb[:], psr[:])
                    nc.vector.tensor_tensor(
                        ct[h // 2][(h % 2) * E:(h % 2) * E + E,
                                   qc * NQ:(qc + 1) * NQ],
                        pso[0:E, :], rb[:], op=MULT)

                def emit_wo_group(qc8, pool=None):
                    # oc=0 / oc=1 chains interleave into two psum banks.
                    pool = pool or psN
                    psAB = [pool.tile([P, NQ], F32, tag=pool.name,
                                      name=f"wo{qc8}_{i}")
                            for i in range(2)]
                    for t in range(T4):
                        for oc in range(2):
                            nc.tensor.matmul(
                                psAB[oc][:],
                                ct[t][:, qc8 * P:(qc8 + 1) * P],
                                wo_sb[t][:, oc * NQ:(oc + 1) * NQ],
                                start=(t == 0), stop=(t == T4 - 1))
                    for oc in range(2):
                        osb = out_pool.tile([P, NQ], OUT16, tag="osb")
                        nc.vector.tensor_copy(osb[:], psAB[oc][:])
                        dma_load(
                            out[qc8 * P:(qc8 + 1) * P, oc * NQ:(oc + 1) * NQ],
                            osb[:])

                iters = [(h, qc) for qc in range(2) for h in range(H)]
                stage_b = None   # (h, qc, p_tiles)
                stage_c = None   # (h, qc, pso, recip_r)
                wo_emitted = 0

                def emit_attn_mm(pso, hprev, kc, pt_pair, start, stop):
                    nc.tensor.matmul(
                        pso[0:E + 1, :],
                        v2[kc][:, hprev, :],
                        pt_pair[kc // 2][:, (kc % 2) * NQ:(kc % 2 + 1) * NQ],
                        start=start, stop=stop)

                for it_idx, (h, qc) in enumerate(iters):
                    t, half = h // 2, h % 2
                    pb = half * E
                    prev = stage_b
                    if stage_c is not None:
                        emit_norm(*stage_c)
                        stage_c = None
                    # qc=0 norms all emitted once it_idx >= 9 (lag-2
                    # pipeline): interleave the qc<4 Wo groups here.
                    if it_idx >= 9 and wo_emitted < 4:
                        emit_wo_group(wo_emitted)
                        wo_emitted += 1
                    pso = None
                    if prev is not None:
                        pso = psO.tile([P, NQ], F32, tag="psO")
                    # interleave: scores (alternating banks of a 2-bank psS
                    # tile) with attnV accumulation of the previous
                    # iteration -- no two consecutive PE matmuls hit the
                    # same psum bank.
                    p_pairs = []
                    for j in range(KC // 2):
                        pss = psS.tile([P, 2 * NQ], F32, tag="psS")
                        for half_j in range(2):
                            kc = 2 * j + half_j
                            nc.tensor.matmul(
                                pss[:, half_j * NQ:(half_j + 1) * NQ],
                                kt[t][pb:pb + E, kc * P:(kc + 1) * P],
                                qt[t][pb:pb + E, qc * NQ:(qc + 1) * NQ],
                                start=True, stop=True,
                                tile_position=(pb, 0))
                            if prev is not None:
                                emit_attn_mm(pso, prev[0], kc, prev[2],
                                             start=(kc == 0),
                                             stop=(kc == KC - 1))
                        pt = p_pool.tile([P, 2 * NQ], F16, tag="pt")
                        nc.scalar.activation(pt[:], pss[:], EXP, scale=0.125)
                        p_pairs.append(pt)
                    if prev is not None:
                        recip_r = emit_recip(pso)
                        stage_c = (prev[0], prev[1], pso, recip_r)
                    stage_b = (h, qc, p_pairs)

                # drain the pipeline
                h, qc, p_pairs = stage_b
                pso = psO.tile([P, NQ], F32, tag="psO")
                for kc in range(KC):
                    emit_attn_mm(pso, h, kc, p_pairs,
                                 start=(kc == 0), stop=(kc == KC - 1))
                recip_r = emit_recip(pso)
                if stage_c is not None:
                    emit_norm(*stage_c)
                emit_norm(h, qc, pso, recip_r)

                # remaining output projection groups; alternate between
                # the psN pool and the (now idle) psS banks so two groups
                # overlap in the drain.
                for i, qc8 in enumerate(range(wo_emitted, KC)):
                    emit_wo_group(qc8, pool=(psS if i % 2 else psN))

    nc.compile()
    return nc


def make_in_maps(queries, keys, values, valid_lens, W_q, W_k, W_v, W_o):
    queries = np.asarray(queries, dtype=np.float32)
    keys = np.asarray(keys, dtype=np.float32)
    values = np.asarray(values, dtype=np.float32)
    valid_lens = np.asarray(valid_lens)
    W_q = np.asarray(W_q, dtype=np.float32)
    W_k = np.asarray(W_k, dtype=np.float32)
    W_v = np.asarray(W_v, dtype=np.float32)
    W_o = np.asarray(W_o, dtype=np.float32)

    if MM_DTYPE == 'bf16':
        import ml_dtypes
        f16 = np.dtype(ml_dtypes.bfloat16)
    else:
        f16 = np.float16
    xtq = [np.ascontiguousarray(queries[b].T.astype(f16)) for b in range(B)]
    xtk = [np.ascontiguousarray(keys[b].T.astype(f16)) for b in range(B)]
    xtv = [np.ascontiguousarray(values[b].T.astype(f16)) for b in range(B)]
    wqt = [np.ascontiguousarray(W_q[hg * DL:(hg + 1) * DL, :].T.astype(f16))
           for hg in range(2)]
    wkt = [np.ascontiguousarray(W_k[hg * DL:(hg + 1) * DL, :].T.astype(f16))
           for hg in range(2)]
    wvt = [np.ascontiguousarray(W_v[hg * DL:(hg + 1) * DL, :].T.astype(f16))
           for hg in range(2)]
    wot = [np.ascontiguousarray(W_o[:, hg * DL:(hg + 1) * DL].T.astype(f16))
           for hg in range(2)]

    in_maps = []
    for c in range(N_CORES):
        b, hg = c // 2, c % 2
        L = int(valid_lens[b])
        k_idx = np.arange(S).reshape(KC, P).T  # [P, KC]
        maskd = (k_idx < L).astype(np.float32)
        in_maps.append({
            "xtq": xtq[b], "xtk": xtk[b], "xtv": xtv[b],
            "wq": wqt[hg], "wk": wkt[hg], "wv": wvt[hg], "wo": wot[hg],
            "maskd": np.ascontiguousarray(maskd),
        })
    return in_maps


def gather(results):
    out = np.empty((B, S, D), dtype=np.float32)
    for b in range(B):
        out[b] = (results[2 * b]["out"].astype(np.float32)
                  + results[2 * b + 1]["out"].astype(np.float32))
    return out


def kernel(queries, keys, values, valid_lens, W_q, W_k, W_v, W_o):
    from concourse.bass_utils import run_bass_kernel_spmd

    if "nc" not in _prog_cache:
        _prog_cache["nc"] = build_program()
    nc = _prog_cache["nc"]

    in_maps = make_in_maps(queries, keys, values, valid_lens,
                           W_q, W_k, W_v, W_o)
    res = run_bass_kernel_spmd(nc, in_maps, core_ids=list(range(N_CORES)))
    return gather(res.results)
